# revision 73
# baseline (speedup 1.0000x reference)
"""Trainium2 Bass kernel for nn_DeepHopfield (self-contained).

Per core (data-parallel over batch: 128 images/core on 8 cores):
  label encoder SHARDED over cores (16 labels/core) -> AllGather(rep
  [16,512] -> [128,512]); hopfield w built from gathered rep; image encoder
  (128 images/core) in single-pass fp16; K Hopfield iterations batch-major
  in fp16 matmuls with fp32 min-energy tracking; two softmax heads in fp32.

conv1 uses a 5-tap replica layout: partitions (5ky, xi-window) so all five
y-taps contract in ONE matmul per (par, og-pair, xq-block) -- half the
column passes of a 4+1 split -- and the whole 2x2 max-pool collapses to a
single vector XY-reduce straight out of PSUM (par and w are trailing dims
of the psum view). conv2 keeps the 4-x-phase replica (A K=128 + B K=64 per
dy) with both y-blocks of an xp sharing each weight slice (repeat-weight
matmuls cost ~30ns vs ~107ns for a weight change since ldw-opt is off);
its pool is an X-reduce over w + partition-shifted ACT copy + one DVE max.

Precision (measured on HW + host-emulated): fp32r matmul operands round to
11-bit RN at 1 cycle/row (4x faster than fp32). The out-head chaotically
amplifies CORRELATED label-branch error: conv WEIGHT rounding at 11 bits
blows the error gate, data rounding is benign. So label conv weights are
f32r hi+lo pairs (~22-bit, 2 passes at 1cyc/row = 2x faster than fp32),
label data/moving operands single f32r, label fc1 stays fp16 hi+lo; the
image branch is single fp16 everywhere.

Scheduling: the Tile scheduler is dependency+priority driven with greedy
backfill, so critical-path chains are kept off busy queues: the scalar
(ACT) queue never carries DMA descriptors, R2 pad-zeroing runs as an ACT
broadcast-copy (label) / late vector memsets (image), the label reshuffle
is high_priority-pinned ahead of the image reshuffle on gpsimd, and both
reshuffles stream out og-pair-incrementally from inside conv1 emission.
"""
import contextlib

import numpy as np

import concourse.bass as bass
import concourse.bass_isa as bass_isa
import concourse.bacc as bacc
import concourse.mybir as mybir
import concourse.tile as tile
from concourse import bass_utils

F32 = mybir.dt.float32
F32R = mybir.dt.float32r
H16 = mybir.dt.float16
AF = mybir.ActivationFunctionType
ALU = mybir.AluOpType

N_CORES = 8
BC = 128          # image batch per core
BL = 16           # label batch per core (label encoder sharded via AllGather)
ITERS = 3         # Hopfield iterations (exact scan converges by 3; min-e tracked)


# ----------------------------------------------------------------- host prep

def _make_replicas5(imgs, b, np_dt=np.float32):
    """[b,1,28,28] -> (R5a [100=(5ky,20xi), 28y*b], R5b [80=(5ky,16xi), 28y*b]).
    Partition (ky, xi) at col (y, bb) holds padded[bb, y+ky, xi0+xi]; all five
    y-taps of the 5x5 conv live in the contraction, so conv1 needs no separate
    5th-tap matmul."""
    assert imgs.shape[0] == b
    pad = np.zeros((b, 32, 32), np.float32)
    pad[:, 2:30, 2:30] = imgs[:, 0]
    outs = []
    for xi0, nxi in ((0, 20), (16, 16)):
        R = np.zeros((5 * nxi, 28 * b), np_dt)
        for ky in range(5):
            sl = pad[:, ky:ky + 28, xi0:xi0 + nxi]        # [b, 28y, nxi]
            R[ky * nxi:(ky + 1) * nxi, :] = \
                np.transpose(sl, (2, 1, 0)).reshape(nxi, 28 * b)
        outs.append(np.ascontiguousarray(R))
    return outs


def _w1n(c1w, par, ogp, blk):
    """conv1 stationary [K=(5ky,nxi), M=(xq,ogip,oc)] for x-parity `par`,
    og-pair `ogp`, xq-block `blk` (0: xq 0-7, 1: xq 8-13)."""
    xi0, nxi, xqs = (0, 20, range(0, 8)) if blk == 0 else (16, 16, range(8, 14))
    Wm = np.zeros((5 * nxi, len(xqs) * 16), np.float32)
    for ky in range(5):
        for xqi, xq in enumerate(xqs):
            for ogip in range(2):
                og = ogp * 2 + ogip
                for dx in range(5):
                    xi_l = 2 * xq + par + dx - xi0
                    if 0 <= xi_l < nxi:
                        Wm[ky * nxi + xi_l,
                           xqi * 16 + ogip * 8: xqi * 16 + ogip * 8 + 8] = \
                            c1w[og * 8:(og + 1) * 8, 0, ky, dx]
    return Wm


def _r11(x):
    """round fp32 to 11-bit mantissa (RN) == the PE's f32r operand rounding"""
    m, e = np.frexp(np.asarray(x, np.float32))
    return np.ldexp(np.round(m * 2**12) / 2**12, e).astype(np.float32)


def _host_prep(inputs):
    """Shared (non-image) constant tensors in device layouts."""
    H = {}
    c1w = np.asarray(inputs['conv1_w'], np.float32)
    c2w = np.asarray(inputs['conv2_w'], np.float32)

    # conv1 stationary blocks: W1NA [100, (par,ogp)*128], W1NB [80, (par,ogp)*96]
    W1NA = np.concatenate([_w1n(c1w, par, ogp, 0)
                           for par in range(2) for ogp in range(2)], axis=1)
    W1NB = np.concatenate([_w1n(c1w, par, ogp, 1)
                           for par in range(2) for ogp in range(2)], axis=1)
    for nm, Wm in (('W1NA', W1NA), ('W1NB', W1NB)):
        H[nm + '_H'] = Wm.astype(np.float16)
        hi = _r11(Wm)
        H[nm + '_RH'] = hi
        H[nm + '_RL'] = (Wm - hi).astype(np.float32)
    # relu bias per partition (xq, ogip, oc), one column per og-pair
    b1 = np.asarray(inputs['conv1_b'], np.float32)
    B1N = np.zeros((128, 2), np.float32)
    for ogp in range(2):
        B1N[:, ogp] = np.tile(np.concatenate([b1[ogp * 16:ogp * 16 + 8],
                                              b1[ogp * 16 + 8:ogp * 16 + 16]]), 8)
    H['B1N'] = B1N

    W2A = np.zeros((5, 128, 128), np.float32)
    W2B = np.zeros((5, 64, 128), np.float32)
    for dy in range(5):
        for j in range(2):
            for xr in range(4):
                dx = xr - j
                if 0 <= dx < 5:
                    W2A[dy, xr * 32:(xr + 1) * 32, j * 64:(j + 1) * 64] = c2w[:, :, dy, dx].T
            for xr2 in range(2):
                dx = 4 + xr2 - j
                if 0 <= dx < 5:
                    W2B[dy, xr2 * 32:(xr2 + 1) * 32, j * 64:(j + 1) * 64] = c2w[:, :, dy, dx].T
    H['W2ASB'] = np.ascontiguousarray(W2A.transpose(1, 0, 2).reshape(128, 640))
    H['W2BSB'] = np.ascontiguousarray(W2B.transpose(1, 0, 2).reshape(64, 640))
    H['B2SB'] = np.tile(np.asarray(inputs['conv2_b'], np.float32), 2)[:, None]

    fw3 = np.asarray(inputs['fc1_w'], np.float32).reshape(512, 64, 7, 7)
    FC1W = np.zeros((28, 128, 512), np.float32)
    for xh in range(4):
        for y in range(7):
            ch = xh * 7 + y
            for par in range(2):
                x = 2 * xh + par
                if x < 7:
                    FC1W[ch, par * 64:(par + 1) * 64, :] = fw3[:, :, y, x].T
    H['FC1B'] = np.ascontiguousarray(np.asarray(inputs['fc1_b'], np.float32).reshape(4, 128).T)
    H['FC1B_BM'] = np.tile(np.asarray(inputs['fc1_b'], np.float32)[None, :], (BL, 1))

    for k in ['W2ASB', 'W2BSB']:
        H[k + '_H'] = H[k].astype(np.float16)
        # f32r hi+lo split for the label branch (~22-bit effective weights)
        hi = _r11(H[k])
        H[k + '_RH'] = hi
        H[k + '_RL'] = (H[k] - hi).astype(np.float32)
    hi = FC1W.astype(np.float16)
    H['FC1W_H'] = hi
    H['FC1W_L'] = (FC1W - hi.astype(np.float32)).astype(np.float16)

    H['FCNW'] = np.ascontiguousarray(
        np.asarray(inputs['fcn_w'], np.float32).T.reshape(4, 128, 128)
        .transpose(1, 0, 2).reshape(128, 512))
    H['FCNB'] = np.tile(np.asarray(inputs['fcn_b'], np.float32)[None, :], (128, 1))

    dm = ((1.0 - np.eye(512, dtype=np.float32)) / 128.0).reshape(4, 128, 512)
    H['DMASK'] = np.ascontiguousarray(dm.transpose(1, 0, 2).reshape(128, 2048)).astype(np.float16)
    H['IDENT'] = np.eye(128, dtype=np.float32)
    return H


# ------------------------------------------------------- device kernel stages

NXB = {0: 5, 2: 4}


def _pool4(nc, dst, s0, s1, s2, s3, tmp):
    """dst = max of 4 PSUM sources via two parallel copy+max chains
    (each op reads at most one PSUM input)."""
    nc.scalar.activation(dst, s0, AF.Copy)
    nc.vector.tensor_tensor(dst, dst, s1, ALU.max)
    nc.scalar.activation(tmp, s2, AF.Copy)
    nc.vector.tensor_tensor(tmp, tmp, s3, ALU.max)
    nc.vector.tensor_tensor(dst, dst, tmp, ALU.max)


def _pool_jw(tc, W, tmpp, ps, nylen, nr, b, par, dst, tag):
    """conv2 2x2 pool: one X-reduce over w (from PSUM), then fold the two
    j partition-halves with a partition-shifted ACT copy + one DVE max."""
    nc = tc.nc
    tmp = tmpp.tile([128, nr * b], H16, tag=tag, name=tag)
    scr = tmpp.tile([128, nr * b], H16, tag=tag + "s", name=tag + "s")
    v = ps[:, 0:nylen].rearrange("p (r w b) -> p r b w", r=nr, w=2)
    nc.vector.tensor_reduce(tmp[:].rearrange("p (r b) -> p r b", r=nr),
                            v, mybir.AxisListType.X, ALU.max)
    p0, p1 = par * 64, (par + 1) * 64
    nc.scalar.activation(scr[p0:p1, :], tmp[64 - p0:128 - p0, :], AF.Copy)
    nc.vector.tensor_tensor(dst, tmp[p0:p1, :].rearrange("p (r b) -> p r b", r=nr),
                            scr[p0:p1, :].rearrange("p (r b) -> p r b", r=nr),
                            ALU.max)


def _conv1_new(tc, W, Ra, Rb, cA, cB, b, wsfx, chunks, psname,
               crA=None, crB=None, psbufs=4, after_chunk=None, after_ogp=None):
    """conv1 via 5-tap replica blocks: per (og-pair, xq-block, y-chunk) one
    K=(5ky,nxi) matmul per parity (all 5 y-taps contracted at once; no
    separate 5th-tap pass). wsfx ('_H',) = fp16 single; ('_RH','_RL') = f32r
    hi+lo. crA/crB: relu destinations (None -> in place)."""
    nc = tc.nc
    with tc.tile_pool(name=psname, bufs=psbufs, space="PSUM") as psum1:
        nblk = 0
        for ogp in range(2):
            for blk, M, wnm, R, c in ((0, 128, 'W1NA', Ra, cA), (1, 96, 'W1NB', Rb, cB)):
                for (y0, ny) in chunks:
                    N = ny * b
                    # par slots bank-aligned at 512 (a matmul output must not
                    # cross a PSUM bank boundary)
                    t = psum1.tile([128, 1024], F32, tag="p1", name="p1ps")
                    for par in range(2):
                        col0 = (par * 2 + ogp) * M
                        for si, sfx in enumerate(wsfx):
                            nc.tensor.matmul(t[0:M, par * 512:par * 512 + N],
                                             W[wnm + sfx][:, col0:col0 + M],
                                             R[:, y0 * b:(y0 + ny) * b],
                                             start=(si == 0), stop=(si == len(wsfx) - 1))
                    nyp = ny // 2
                    dst = c[0:M, :].rearrange("p (g y b) -> p g y b", g=2, y=14)[
                        :, ogp, y0 // 2: y0 // 2 + nyp, :]
                    # whole 2x2 max-pool in ONE vector op: reduce (par, w)
                    v = t[0:M, :].rearrange("p (par c) -> p par c", par=2)[:, :, 0:N] \
                        .rearrange("p par (y w b) -> p y b par w", y=nyp, w=2)
                    nc.vector.tensor_reduce(dst, v, mybir.AxisListType.XY, ALU.max)
                    nblk += 1
                    if after_chunk is not None:
                        after_chunk(nblk)
            for c, cr, M in ((cA, crA, 128), (cB, crB, 96)):
                sl = c[0:M, :].rearrange("p (g c) -> p g c", g=2)[:, ogp, :]
                dstr = sl if cr is None else \
                    cr[0:M, :].rearrange("p (g c) -> p g c", g=2)[:, ogp, :]
                nc.scalar.activation(dstr, sl, AF.Relu,
                                     bias=W['B1N'][0:M, ogp:ogp + 1])
            if after_ogp is not None:
                after_ogp(ogp)


def _reshuffle(tc, cA, cB, b, R2, engines=None, ogs=(0, 1, 2, 3)):
    """(cA, cB) -> conv2 x-phase replica tiles (pads pre-zeroed by caller).
    cA holds xq 0-7, cB holds xq 8-13, partitions (xq, ogip, oc); `ogs`
    restricts emission to a subset of og blocks so the caller can interleave."""
    nc = tc.nc
    engines = engines or [nc.sync]
    i = 0
    for og in ogs:
        ogp, ogip = og // 2, og % 2
        for xbp in range(5):
            for psi in (0, 2):
                if xbp >= NXB[psi]:
                    continue
                for xr in range(4):
                    xp = psi + 4 * xbp + xr - 2
                    if not (0 <= xp < 14):
                        continue
                    if xp < 8:
                        src = cA[xp * 16 + ogip * 8: xp * 16 + ogip * 8 + 8,
                                 ogp * 14 * b:(ogp + 1) * 14 * b]
                    else:
                        src = cB[(xp - 8) * 16 + ogip * 8: (xp - 8) * 16 + ogip * 8 + 8,
                                 ogp * 14 * b:(ogp + 1) * 14 * b]
                    engines[i % len(engines)].dma_start(
                        R2[psi][xr * 32 + og * 8: xr * 32 + (og + 1) * 8,
                                xbp * 18 * b + 2 * b: xbp * 18 * b + 16 * b],
                        src)
                    i += 1
    return R2


def _conv2_image(tc, W, R2, pooled2):
    nc = tc.nc
    b = BC
    with tc.tile_pool(name="p2tmpI", bufs=2) as tmpp, \
         tc.tile_pool(name="psum2I", bufs=4, space="PSUM") as psum2:
        for xp in range(7):
            psi = (2 * xp) % 4
            xb = (2 * xp - psi) // 4
            par, xh = xp % 2, xp // 2
            # both y-blocks of one xp live together so each weight slice is
            # loaded once and reused by 4 consecutive matmuls (repeat-weights
            # matmuls cost ~30ns extra vs ~107ns for a weight change)
            T = {y0: psum2.tile([128, 8 * b], F32, tag="p2", name="p2ps")
                 for y0 in (0, 8)}
            segs = ((0, 0, 512), (0, 512, 512), (8, 0, 512), (8, 512, 256))
            for dy in range(5):
                for rows, wname, xbb in ((128, 'W2ASB_H', xb), (64, 'W2BSB_H', xb + 1)):
                    lw = W[wname][:, dy * 128:(dy + 1) * 128]
                    for (y0, lo, n) in segs:
                        base = (xbb * 18 + y0 + dy) * b
                        nc.tensor.matmul(T[y0][:, lo:lo + n], lw,
                                         R2[psi][0:rows, base + lo: base + lo + n],
                                         start=(dy == 0 and rows == 128),
                                         stop=(dy == 4 and rows == 64))
            for (y0, ny) in ((0, 8), (8, 6)):
                nylen = ny * b
                ps = T[y0]
                nr = ny // 2
                _pool_jw(tc, W, tmpp, ps, nylen, nr, b, par,
                         pooled2[par * 64:(par + 1) * 64,
                                 xh * 7 * b + (y0 // 2) * b:
                                 xh * 7 * b + (y0 // 2 + nr) * b]
                         .rearrange("p (r b) -> p r b", r=nr), "p2tmp")
    nc.gpsimd.memset(pooled2[64:128, 3 * 7 * b:4 * 7 * b], 0.0)
    for xh in range(4):
        sl = pooled2[:, xh * 7 * b:(xh + 1) * 7 * b]
        nc.scalar.activation(sl, sl, AF.Relu, bias=W['B2SB'][:, 0:1])
    return pooled2


def _fc1_image(tc, cpool, W, pooled2):
    nc = tc.nc
    b = BC
    outs = []
    with tc.tile_pool(name="fc1sI", bufs=1) as fc1sp, \
         tc.tile_pool(name="psum3I", bufs=1, space="PSUM") as psum3:
        lat_bm = psum3.tile([128, 512], F32, tag="latbm", name="lat_bm")
        for ch in range(28):
            nc.tensor.matmul(lat_bm[:], pooled2[:, ch * b:(ch + 1) * b],
                             W['FC1WH'][:, ch * 512:(ch + 1) * 512],
                             start=(ch == 0), stop=(ch == 27))
        lat_sb = fc1sp.tile([128, 512], F32, name="lat_sbI")
        nc.scalar.activation(lat_sb[:], lat_bm[:], AF.Copy)
        for lt in range(4):
            tp = psum3.tile([128, 128], F32, tag="latT", name="lat_tp", bufs=2)
            nc.tensor.transpose(tp[:], lat_sb[:, lt * 128:(lt + 1) * 128], W['IDENT'][:])
            o = cpool.tile([128, b], F32, tag=f"encI{lt}", name=f"encI{lt}")
            nc.scalar.activation(o[:], tp[:], AF.Identity, bias=W['FC1B'][:, lt:lt + 1])
            outs.append(o)
    return outs





LC2_GROUPS = [(0, 0, 2), (0, 2, 2), (2, 0, 2), (2, 2, 1)]  # (psi, xb0, n)


class LabelConv2:
    """Label conv2 in f32r hi+lo: contiguous pair-window matmuls (N=512 per
    2-xb group, garbage in the inter-block gap cols). Emission is chunked so
    the caller can interleave the 20 matmul steps into image-conv1 pipeline
    bubbles."""

    def __init__(self, tc, W, R2, pooled2, ctx):
        self.tc, self.W, self.R2, self.pooled2 = tc, W, R2, pooled2
        self.tmpp = ctx.enter_context(tc.tile_pool(name="p2tmpL", bufs=2))
        self.psum2 = ctx.enter_context(tc.tile_pool(name="psum2L", bufs=2, space="PSUM"))
        self.gi = 0

    def emit_group(self):
        nc = self.tc.nc
        b = BL
        W18 = 18 * b
        psi, xb0, n = LC2_GROUPS[self.gi]
        N = 512 if n == 2 else 224
        ps = self.psum2.tile([128, 512], F32, tag="p2L", name="p2L")
        wdefs = ((('W2ASB_RH', 'W2ASB_RL'), 128, 0), (('W2BSB_RH', 'W2BSB_RL'), 64, 1))
        for dy in range(5):
            for wi, (wnames, rows, xoff) in enumerate(wdefs):
                for hl, wname in enumerate(wnames):
                    lw = self.W[wname][:, dy * 128:(dy + 1) * 128]
                    base = (xb0 + xoff) * W18 + dy * b
                    nc.tensor.matmul(
                        ps[:, 0:N], lw, self.R2[psi][0:rows, base:base + N],
                        start=(dy == 0 and wi == 0 and hl == 0),
                        stop=(dy == 4 and wi == 1 and hl == 1))
        for i in range(n):
            xb = xb0 + i
            xp = 2 * xb + psi // 2
            par, xh = xp % 2, xp // 2
            _pool_jw(self.tc, self.W, self.tmpp,
                     ps[:, i * W18: i * W18 + 14 * b], 14 * b, 7, b,
                     par,
                     self.pooled2[par * 64:(par + 1) * 64,
                                  xh * 7 * b:(xh + 1) * 7 * b]
                     .rearrange("p (r b) -> p r b", r=7), "p2tmpL")
        self.gi += 1

    def finish(self):
        nc = self.tc.nc
        b = BL
        while self.gi < len(LC2_GROUPS):
            self.emit_group()
        nc.gpsimd.memset(self.pooled2[64:128, 3 * 7 * b:4 * 7 * b], 0.0)
        nc.scalar.activation(self.pooled2[:], self.pooled2[:], AF.Relu,
                             bias=self.W['B2SB'][:, 0:1])


def _fc1_label(tc, W, pooled2, rep_sh):
    nc = tc.nc
    b = BL
    with tc.tile_pool(name="fc1L", bufs=1) as fcp, \
         tc.tile_pool(name="psum3L", bufs=1, space="PSUM") as psum3:
        p16 = fcp.tile([128, 4 * 7 * b], H16, name="p16L")
        nc.scalar.activation(p16[:], pooled2[:], AF.Copy)
        lat_bm = psum3.tile([BL, 512], F32, tag="latbmL", name="lat_bmL")
        for ch in range(28):
            st = p16[:, ch * b:(ch + 1) * b]
            nc.tensor.matmul(lat_bm[:], st, W['FC1WH'][:, ch * 512:(ch + 1) * 512],
                             start=(ch == 0), stop=False)
            nc.tensor.matmul(lat_bm[:], st, W['FC1WL'][:, ch * 512:(ch + 1) * 512],
                             start=False, stop=(ch == 27))
        pre = fcp.tile([BL, 512], F32, name="rep_pre")
        nc.vector.tensor_tensor(pre[:], lat_bm[:], W['FC1B_BM'][:], ALU.add)
        nc.scalar.activation(rep_sh[:], pre[:], AF.Tanh)


def _softmax_head(tc, vpool, cps, tag, logits_fn, dst):
    nc = tc.nc
    lg_ps = cps.tile([128, 128], F32, tag=f"lg_{tag}", name=f"lg_{tag}")
    logits = logits_fn(lg_ps)
    mx = vpool.tile([128, 1], F32, tag=f"mx{tag}", name="mx")
    nc.vector.tensor_reduce(mx[:], logits[:], mybir.AxisListType.X, ALU.max)
    mxn = vpool.tile([128, 1], F32, tag=f"mxn{tag}", name="mxn")
    nc.vector.tensor_scalar(mxn[:], mx[:], -1.0, None, ALU.mult)
    ex = vpool.tile([128, 128], F32, tag=f"ex{tag}", name="ex")
    nc.scalar.activation(ex[:], logits[:], AF.Exp, bias=mxn[:])
    sme = vpool.tile([128, 1], F32, tag=f"sme{tag}", name="sme")
    nc.vector.tensor_reduce(sme[:], ex[:], mybir.AxisListType.X, ALU.add)
    rec = vpool.tile([128, 1], F32, tag=f"rec{tag}", name="rec")
    nc.vector.reciprocal(rec[:], sme[:])
    prob = vpool.tile([128, 128], F32, tag=f"prob{tag}", name="prob")
    nc.vector.tensor_scalar(prob[:], ex[:], rec[:], None, ALU.mult)
    nc.sync.dma_start(dst[:], prob[:])


def build_program():
    """Build the full Bass program; returns (nc, input_names, output_names)."""
    nc = bacc.Bacc("TRN2", target_bir_lowering=False, debug=False, num_devices=N_CORES)
    b = BC

    din = {}
    def dram_in(name, shape, dt=F32):
        din[name] = nc.dram_tensor(name, list(shape), dt, kind="ExternalInput").ap()

    for name, shape in [('B1N', (128, 2)), ('B2SB', (128, 1)),
                        ('FC1B', (128, 4)), ('FC1B_BM', (BL, 512)),
                        ('FCNW', (128, 512)), ('FCNB', (128, 128)),
                        ('IDENT', (128, 128))]:
        dram_in(name, shape)
    for name, shape in [('R1LA', (100, 28 * BL)), ('R1LB', (80, 28 * BL)),
                        ('W1NA_RH', (100, 512)), ('W1NA_RL', (100, 512)),
                        ('W1NB_RH', (80, 384)), ('W1NB_RL', (80, 384)),
                        ('W2ASB_RH', (128, 640)), ('W2ASB_RL', (128, 640)),
                        ('W2BSB_RH', (64, 640)), ('W2BSB_RL', (64, 640))]:
        dram_in(name, shape, mybir.dt.float32r)
    dram_in('DMASK', (128, 2048), H16)
    for name, shape in [('R1A', (100, 28 * BC)), ('R1B', (80, 28 * BC)),
                        ('W1NA_H', (100, 512)), ('W1NB_H', (80, 384)),
                        ('W2ASB_H', (128, 640)), ('W2BSB_H', (64, 640)),
                        ('FC1W_H', (28, 128, 512)), ('FC1W_L', (28, 128, 512))]:
        dram_in(name, shape, H16)
    out_d = nc.dram_tensor('OUT', [128, 128], F32, kind="ExternalOutput").ap()
    lbl_d = nc.dram_tensor('LABEL', [128, 128], F32, kind="ExternalOutput").ap()

    with tile.TileContext(nc) as tc, contextlib.ExitStack() as ctx:
        wpool = ctx.enter_context(tc.tile_pool(name="weights", bufs=1))
        cpool = ctx.enter_context(tc.tile_pool(name="persist", bufs=1))
        dramp = ctx.enter_context(tc.tile_pool(name="dram", bufs=1, space="DRAM"))

        # encoder working tiles; created before the weight DMAs so the replica
        # loads lead the scalar queue (pools close LIFO: image, label, RI)
        ectxI = ctx.enter_context(contextlib.ExitStack())
        ipool = ectxI.enter_context(tc.tile_pool(name="imgbufs", bufs=1))
        c1pIA = ipool.tile([128, 2 * 14 * BC], H16, name="c1pIA")
        c1pIB = ipool.tile([96, 2 * 14 * BC], H16, name="c1pIB")
        R2I = {psi: ipool.tile([128, NXB[psi] * 18 * BC], H16, name=f"r2_{psi}I")
               for psi in (0, 2)}
        pooled2I = ipool.tile([128, 4 * 7 * BC], H16, name="pooled2I")
        ectxL = contextlib.ExitStack()
        lpool = ectxL.enter_context(tc.tile_pool(name="lblbufs", bufs=1))
        RLa = lpool.tile([100, 28 * BL], F32R, name="RLa")
        RLb = lpool.tile([80, 28 * BL], F32R, name="RLb")
        nc.scalar.dma_start(RLa[:], din['R1LA'][:])
        nc.scalar.dma_start(RLb[:], din['R1LB'][:])
        rep_sh = lpool.tile([BL, 512], F32, name="rep_sh")
        c1pLA = lpool.tile([128, 2 * 14 * BL], H16, name="c1pLA")
        c1pLB = lpool.tile([96, 2 * 14 * BL], H16, name="c1pLB")
        c1pLrA = lpool.tile([128, 2 * 14 * BL], F32R, name="c1pLrA")
        c1pLrB = lpool.tile([96, 2 * 14 * BL], F32R, name="c1pLrB")
        R2L = {psi: lpool.tile([128, NXB[psi] * 18 * BL], F32R, name=f"r2_{psi}L")
               for psi in (0, 2)}
        pooled2L = lpool.tile([128, 4 * 7 * BL], F32, name="pooled2L")
        W = {}
        # lc2's pools must sit below repl_I on the pool stack (repl_I closes
        # first); its W references are populated by the load loop below.
        lc2 = LabelConv2(tc, W, R2L, pooled2L, ectxL)
        rstackI = contextlib.ExitStack()
        rpoolI = rstackI.enter_context(tc.tile_pool(name="repl_I", bufs=1))
        RIa = rpoolI.tile([100, 28 * BC], H16, name="RIa")
        RIb = rpoolI.tile([80, 28 * BC], H16, name="RIb")
        for k in range(4):
            nc.sync.dma_start(RIa[:, k * 896:(k + 1) * 896],
                              din['R1A'][:, k * 896:(k + 1) * 896])
            nc.sync.dma_start(RIb[:, k * 896:(k + 1) * 896],
                              din['R1B'][:, k * 896:(k + 1) * 896])

        # first-needed tensors issue from otherwise-idle engines (sync-queue
        # DMA issue is serialized at ~0.15us per descriptor)
        for eng, name, shape, dt in [
                (nc.scalar, 'W1NA_RH', (100, 512), F32R),
                (nc.scalar, 'W1NA_RL', (100, 512), F32R),
                (nc.scalar, 'W1NB_RH', (80, 384), F32R),
                (nc.scalar, 'W1NB_RL', (80, 384), F32R),
                (nc.scalar, 'B1N', (128, 2), F32),
                (nc.gpsimd, 'W1NA_H', (100, 512), H16),
                (nc.gpsimd, 'W1NB_H', (80, 384), H16),
                (nc.gpsimd, 'W2ASB_RH', (128, 640), F32R),
                (nc.gpsimd, 'W2ASB_RL', (128, 640), F32R),
                (nc.gpsimd, 'W2BSB_RH', (64, 640), F32R),
                (nc.gpsimd, 'W2BSB_RL', (64, 640), F32R),
                (nc.gpsimd, 'W2ASB_H', (128, 640), H16),
                (nc.gpsimd, 'W2BSB_H', (64, 640), H16),
                (nc.gpsimd, 'B2SB', (128, 1), F32),
                (nc.gpsimd, 'FC1B', (128, 4), F32),
                (nc.gpsimd, 'FC1B_BM', (BL, 512), F32)]:
            t = wpool.tile(list(shape), dt, tag=name, name=name)
            eng.dma_start(t[:], din[name][:])
            W[name] = t
        for nm, srcnm in (('FC1WH', 'FC1W_H'), ('FC1WL', 'FC1W_L')):
            t = wpool.tile([128, 28 * 512], H16, tag=nm, name=nm)
            # 4 descriptors each: a single 3.7MB descriptor blocks the sync
            # queue for ~10-14us of issue time
            for q in range(4):
                nc.sync.dma_start(
                    t[:, q * 7 * 512:(q + 1) * 7 * 512].rearrange(
                        "p (ch c) -> p ch c", ch=7),
                    din[srcnm][q * 7:(q + 1) * 7, :, :].rearrange("ch p c -> p ch c"))
            W[nm] = t
        for name, shape, dt in [('FCNW', (128, 512), F32), ('FCNB', (128, 128), F32),
                                ('DMASK', (128, 2048), H16), ('IDENT', (128, 128), F32)]:
            t = wpool.tile(list(shape), dt, tag=name, name=name)
            nc.sync.dma_start(t[:], din[name][:])
            W[name] = t
        ident16 = wpool.tile([128, 128], H16, tag="ident16", name="ident16")
        nc.vector.tensor_copy(ident16[:], W['IDENT'][:])

        rep_nat = cpool.tile([128, 512], F32, tag="rep_nat", name="rep_nat")

        # ---- interleaved label/image encoder emission ----
        if True:
            # zero R2L pads on the ACT engine (early, no DMA-queue coupling)
            zcol = lpool.tile([128, 1], F32, name="zcol")
            nc.vector.memset(zcol[:], 0.0)
            for psi in (0, 2):
                nc.scalar.activation(R2L[psi][:],
                                     zcol[:].broadcast_to([128, NXB[psi] * 18 * BL]),
                                     AF.Copy)
            def _lresh(ogp):
                # pin ahead of the image reshuffle in the gpsimd queue order
                # (deps still enforced; only the scheduler's heap order changes)
                with tc.high_priority():
                    _reshuffle(tc, c1pLrA, c1pLrB, BL, R2L,
                               engines=[nc.gpsimd], ogs=(2 * ogp, 2 * ogp + 1))

            _conv1_new(tc, W, RLa, RLb, c1pLA, c1pLB, BL, ('_RH', '_RL'),
                       [(0, 28)], "ps1L", crA=c1pLrA, crB=c1pLrB, psbufs=2,
                       after_ogp=_lresh)
            for psi in (0, 2):
                # after the label reduces so they don't head-of-line block them
                nc.vector.memset(R2I[psi][:], 0.0)

            def after_ogp(ogp):
                # stream the conv2 replica shuffle out while conv1 still runs
                _reshuffle(tc, c1pIA, c1pIB, BC, R2I,
                           engines=[nc.sync, nc.gpsimd], ogs=(2 * ogp, 2 * ogp + 1))

            _conv1_new(tc, W, RIa, RIb, c1pIA, c1pIB, BC, ('_H',),
                       [(y0, 4) for y0 in range(0, 28, 4)], "ps1I",
                       psbufs=3, after_ogp=after_ogp)
            rstackI.close()                                # free RI before conv2
            lc2.finish()
            _fc1_label(tc, W, pooled2L, rep_sh)
            ag_in = dramp.tile([BL, 512], F32, name="ag_in")
            ag_out = dramp.tile([128, 512], F32, name="ag_out")
            nc.gpsimd.dma_start(ag_in[:], rep_sh[:])
            ectxL.close()                                  # free label pools
            nc.gpsimd.collective_compute(
                "AllGather", mybir.AluOpType.bypass,
                replica_groups=[list(range(N_CORES))],
                ins=[ag_in.opt()], outs=[ag_out.opt()])
            nc.gpsimd.dma_start(rep_nat[:], ag_out[:])
            _conv2_image(tc, W, R2I, pooled2I)             # AllGather hides here
            # rho and tB on vector/gpsimd only -- overlaps image fc1
            rsum = cpool.tile([128, 1], F32, tag="rsum", name="rsum")
            nc.vector.tensor_reduce(rsum[:], rep_nat[:], mybir.AxisListType.X, ALU.add)
            rho_all = cpool.tile([128, 1], F32, tag="rho_all", name="rho_all")
            nc.gpsimd.partition_all_reduce(rho_all[:], rsum[:], 128,
                                           bass_isa.ReduceOp.add)
            rho_col = cpool.tile([128, 1], F32, tag="rho_col", name="rho_col")
            nc.vector.tensor_scalar(rho_col[:], rho_all[:], 1.0 / 65536.0, None, ALU.mult)
            tB = cpool.tile([128, 512], F32, tag="tB", name="tB")
            nc.vector.tensor_scalar(tB[:], rep_nat[:], rho_col[:], None, ALU.subtract)
            latT = _fc1_image(tc, cpool, W, pooled2I)

        # label head early: its vector/scalar chain overlaps w-build+clustering
        with tc.tile_pool(name="lblh", bufs=1) as vpoolh, \
             tc.tile_pool(name="lblh_ps", bufs=1, space="PSUM") as cpsh:
            def _lbl_logits(lg_ps):
                for k in range(4):
                    nc.tensor.matmul(lg_ps[:], latT[k][:],
                                     W['FCNW'][:, k * 128:(k + 1) * 128],
                                     start=(k == 0), stop=(k == 3))
                logits = vpoolh.tile([128, 128], F32, tag="lgs2", name="lgs2")
                nc.vector.tensor_tensor(logits[:], lg_ps[:], W['FCNB'][:], ALU.add)
                return logits
            _softmax_head(tc, vpoolh, cpsh, 'label', _lbl_logits, lbl_d)

        # ---- hopfield w (from gathered rep_nat [128 lbl, 512 lat], fp32) ----
        w16 = cpool.tile([128, 2048], H16, tag="w16", name="w16")
        repT = []
        with tc.tile_pool(name="wb_ps", bufs=1, space="PSUM") as pp:
            for jc in range(4):
                w_ps = pp.tile([128, 512], F32, tag="wps", name="w_ps", bufs=2)
                nc.tensor.matmul(w_ps[:], tB[:, jc * 128:(jc + 1) * 128], tB[:],
                                 start=True, stop=True)
                nc.vector.tensor_tensor(w16[:, jc * 512:(jc + 1) * 512], w_ps[:],
                                        W['DMASK'][:, jc * 512:(jc + 1) * 512], ALU.mult)
            for k in range(4):
                tp = pp.tile([128, 128], F32, tag="repT", name="repT_ps", bufs=2)
                nc.tensor.transpose(tp[:], rep_nat[:, k * 128:(k + 1) * 128], W['IDENT'][:])
                rt = cpool.tile([128, 128], F32, tag=f"repT{k}", name=f"repT{k}")
                nc.scalar.activation(rt[:], tp[:], AF.Copy)
                repT.append(rt)

        # ---- clustering: batch-major fp16 matmuls, fp32 min tracking ----
        with tc.tile_pool(name="clv", bufs=2) as vpool, \
             tc.tile_pool(name="cl_ps", bufs=1, space="PSUM") as cps:
            s16 = []
            for k in range(4):
                t = cpool.tile([128, b], H16, tag=f"s16_{k}", name=f"s16_{k}")
                nc.scalar.activation(t[:], latT[k][:], AF.Tanh)
                s16.append(t)
            smag_bm = cpool.tile([128, 512], H16, tag="smag_bm", name="smag_bm")
            for k in range(4):
                tp = cps.tile([128, 128], H16, tag="sT", name="sT_ps", bufs=2)
                nc.tensor.transpose(tp[:], s16[k][:], ident16[:])
                nc.scalar.activation(smag_bm[:, k * 128:(k + 1) * 128], tp[:], AF.Abs)
            min_e = cpool.tile([128, 1], F32, tag="min_e", name="min_e")
            nc.vector.memset(min_e[:], 3.0e38)
            min_s_bm = cpool.tile([128, 512], F32, tag="min_s_bm", name="min_s_bm")
            nc.vector.memset(min_s_bm[:], 0.0)

            def mm_h16(src):
                ps = cps.tile([128, 512], F32, tag="h", name="h_ps", bufs=2)
                for jc in range(4):
                    nc.tensor.matmul(ps[:], src[jc][:], w16[:, jc * 512:(jc + 1) * 512],
                                     start=(jc == 0), stop=(jc == 3))
                return ps

            h = mm_h16(s16)
            for it in range(ITERS):
                sg = vpool.tile([128, 512], H16, tag="sg", name="sg")
                nc.scalar.activation(sg[:], h[:], AF.Sign)
                sn_bm = vpool.tile([128, 512], H16, tag="sn_bm", name="sn_bm")
                nc.vector.tensor_tensor(sn_bm[:], smag_bm[:], sg[:], ALU.mult)
                snew = []
                for k in range(4):
                    tp = cps.tile([128, 128], H16, tag="sT", name="sT_ps", bufs=2)
                    nc.tensor.transpose(tp[:], sn_bm[:, k * 128:(k + 1) * 128], ident16[:])
                    t = vpool.tile([128, b], H16, tag=f"sn{k}", name=f"sn{k}")
                    nc.scalar.activation(t[:], tp[:], AF.Copy)
                    snew.append(t)
                h = mm_h16(snew)
                pr = vpool.tile([128, 512], F32, tag="pr", name="pr")
                nc.vector.tensor_tensor(pr[:], h[:], sn_bm[:], ALU.mult)
                e_col = vpool.tile([128, 1], F32, tag="ecol", name="e_col")
                nc.vector.tensor_reduce(e_col[:], pr[:], mybir.AxisListType.X, ALU.add)
                nc.vector.tensor_scalar(e_col[:], e_col[:], -1.0, None, ALU.mult)
                mask = vpool.tile([128, 1], F32, tag="mask", name="mask")
                nc.vector.tensor_tensor(mask[:], e_col[:], min_e[:], ALU.is_lt)
                mask_i = vpool.tile([128, 1], mybir.dt.int32, tag="mask_i", name="mask_i")
                nc.vector.tensor_copy(mask_i[:], mask[:])
                nc.vector.copy_predicated(min_e[:], mask_i[:], e_col[:])
                d1 = vpool.tile([128, 512], F32, tag="d1", name="d1")
                nc.vector.tensor_tensor(d1[:], sn_bm[:], min_s_bm[:], ALU.subtract)
                nc.vector.tensor_scalar(d1[:], d1[:], mask[:], None, ALU.mult)
                nc.vector.tensor_tensor(min_s_bm[:], min_s_bm[:], d1[:], ALU.add)

            min_s = []
            for k in range(4):
                tp = cps.tile([128, 128], F32, tag="msT", name="msT_ps", bufs=2)
                nc.tensor.transpose(tp[:], min_s_bm[:, k * 128:(k + 1) * 128], W['IDENT'][:])
                t = vpool.tile([128, 128], F32, tag=f"ms{k}", name=f"ms{k}")
                nc.scalar.activation(t[:], tp[:], AF.Copy)
                min_s.append(t)

            # ---- out head ----
            def _out_logits(lg_ps):
                for k in range(4):
                    nc.tensor.matmul(lg_ps[:], min_s[k][:], repT[k][:],
                                     start=(k == 0), stop=(k == 3))
                logits = vpool.tile([128, 128], F32, tag="lgs", name="lgs")
                nc.scalar.activation(logits[:], lg_ps[:], AF.Abs)
                return logits
            _softmax_head(tc, vpool, cps, 'out', _out_logits, out_d)

    nc.compile()
    in_names = list(din.keys())
    return nc, in_names, ['OUT', 'LABEL']


# --------------------------------------------------------------- entry point

_CACHE = {}
TRACE = False     # set True (e.g. from test.py) to capture a neuron profile


def kernel(**inputs):
    if 'prog' not in _CACHE:
        _CACHE['prog'] = build_program()
    nc, in_names, out_names = _CACHE['prog']

    H = _host_prep(inputs)
    image = np.asarray(inputs['image'], np.float32)
    labels = np.asarray(inputs['label_images'], np.float32)
    shared = {k: H[k] for k in
              ['B1N', 'B2SB',
               'FC1B', 'FC1B_BM', 'FCNW', 'FCNB', 'DMASK', 'IDENT',
               'W1NA_H', 'W1NB_H', 'W2ASB_H', 'W2BSB_H',
               'W1NA_RH', 'W1NA_RL', 'W1NB_RH', 'W1NB_RL',
               'W2ASB_RH', 'W2ASB_RL', 'W2BSB_RH', 'W2BSB_RL',
               'FC1W_H', 'FC1W_L']}
    in_maps = []
    for c in range(N_CORES):
        m = dict(shared)
        m['R1A'], m['R1B'] = _make_replicas5(image[c * BC:(c + 1) * BC], BC, np.float16)
        m['R1LA'], m['R1LB'] = _make_replicas5(labels[c * BL:(c + 1) * BL], BL)
        in_maps.append(m)

    res = bass_utils.run_bass_kernel_spmd(nc, in_maps, core_ids=list(range(N_CORES)),
                                          trace=TRACE)
    _CACHE['last_results'] = res
    outs = np.concatenate([res.results[c]['OUT'] for c in range(N_CORES)], axis=0)
    labels_o = np.concatenate([res.results[c]['LABEL'] for c in range(N_CORES)], axis=0)
    return outs, labels_o

